# revision 1
# baseline (speedup 1.0000x reference)
"""DSBlock Trainium2 kernel — data-parallel over batch (1 sample / NeuronCore).

Per-sample pipeline (per core):
  local:  3x3 conv (6-pass shifted-matmul, scale folded in weights) -> relu ->
          qkv (head-padded layouts) -> cross attn with pooled-global k/v
  global: 4x4 mean pool -> qkv -> cross attn with local k/v -> bilinear resize
          (dense resize matmul with host-built matrix)
  merge:  1x1 conv (scale folded) + bias + relu

Engine plan: ACT does nothing but softmax exp (the throughput floor);
DVE does psum evictions/bias/relu/pool; PE does all matmuls with
4-head tile_position packing (head_dim=16 -> 32-row/col array strips).
Softmax denominators ride the attn@v matmuls as an extra ones-column;
normalization happens post-contraction via reciprocal + PE broadcast.
"""

import sys

if "/opt/trn_rl_repo" not in sys.path:
    sys.path.insert(0, "/opt/trn_rl_repo")

from contextlib import ExitStack

import numpy as np

import concourse.bacc as bacc
import concourse.bass as bass
import concourse.tile as tile
from concourse import mybir
from concourse.bass_utils import run_bass_kernel_spmd

F32 = mybir.dt.float32
F16 = mybir.dt.float16
ADD = mybir.AluOpType.add
MAX = mybir.AluOpType.max
MULT = mybir.AluOpType.mult
EXP = mybir.ActivationFunctionType.Exp

B, C, H, W = 8, 128, 64, 64
CH = C // 2          # 64
HS, WS = 16, 16
HEADS = 4
HD = CH // HEADS     # 16
N = H * W            # 4096 local tokens
S = HS * WS          # 256 global tokens
SCALE = 0.25         # 1/sqrt(HD)


def _resize_matrix():
    """R1[o, i]: 16 -> 64 bilinear, half-pixel centers, edge clamp."""
    R1 = np.zeros((64, 16), np.float64)
    for o in range(64):
        c = (o + 0.5) / 4.0 - 0.5
        i0 = int(np.floor(c))
        w1 = c - i0
        i0c = min(max(i0, 0), 15)
        i1c = min(max(i0 + 1, 0), 15)
        R1[o, i0c] += 1.0 - w1
        R1[o, i1c] += w1
    return R1


def _emit(ctx, tc, nc, d):
    import os
    STAGE = int(os.environ.get("KSTAGE", "9"))
    ts_ = bass.ts

    def _finish(src_ap, rows, cols):
        nc.gpsimd.memset(OUTS[:], 0.0)
        nc.vector.tensor_copy(OUTS[0:rows, 0:cols], src_ap)
        nc.sync.dma_start(out=d["out"][:], in_=OUTS[:])

    consts = ctx.enter_context(tc.tile_pool(name="consts", bufs=1))
    feat = ctx.enter_context(tc.tile_pool(name="feat", bufs=1))
    ppool_cm = tc.tile_pool(name="ppool", bufs=3, space="PSUM")
    ppool = ppool_cm.__enter__()
    epool = ctx.enter_context(tc.tile_pool(name="epool", bufs=6))

    def load(name, shape, dtype, pool=consts, tag=None):
        t = pool.tile(shape, dtype, tag=tag or name, name=tag or name)
        nc.sync.dma_start(out=t[:], in_=d[name][:])
        return t

    # ---- constants / inputs -------------------------------------------------
    XP = consts.tile([128, 66 * 66], F16, tag="xp", name="xp")
    for qq in range(4):
        nc.sync.dma_start(out=XP[:, 1089 * qq:1089 * (qq + 1)],
                          in_=d["xp"][:, 1089 * qq:1089 * (qq + 1)])
    XG = load("xg", [64, 64 * 64], F16)
    DWP = load("dwp", [128, 192], F16)
    DWS = load("dws", [64, 192], F16)
    DEPB = load("depb", [64, 1], F32)
    WQKT = load("wqkt", [65, 256], F16)
    WVT = load("wvt", [65, 128], F16)
    LWT = load("lwt", [128, 256], F16)
    LB = load("lb", [128, 1], F32)
    SEL4 = load("sel4", [4, 128], F16)
    IDENT = load("ident", [128, 128], F16)
    R2 = [consts.tile([128, 4096], F16, tag=f"r2_{c}", name=f"r2_{c}") for c in range(2)]
    for c in range(2):
        nc.sync.dma_start(out=R2[c][:], in_=d["r2dt"][128 * c:128 * (c + 1), :])

    # ---- feature buffers ----------------------------------------------------
    FLA = feat.tile([65, 4096], F16, tag="fla", name="fla")     # local feats + ones row
    FGA = feat.tile([65, 256], F16, tag="fga", name="fga")      # global feats + ones row
    LQ = feat.tile([128, 4096], F16, tag="lq", name="lq")      # head-padded (32h+d)
    LK = feat.tile([128, 4096], F16, tag="lk", name="lk")
    LV = feat.tile([128, 4096], F16, tag="lv", name="lv")      # token-major [tok, 128c+32h+d]
    GQ = feat.tile([128, 256], F16, tag="gq", name="gq")
    GK = feat.tile([128, 256], F16, tag="gk", name="gk")
    GV = feat.tile([128, 256], F16, tag="gv", name="gv")       # token-major
    FLGR = feat.tile([128, 4096], F16, tag="flgr", name="flgr")  # raw local-attn out
    FLGN = feat.tile([128, 4096], F16, tag="flgn", name="flgn")  # normalized
    FGLS = feat.tile([128, 256], F16, tag="fgls", name="fgls")
    FGLN = feat.tile([128, 256], F16, tag="fgln", name="fgln")
    FGLT = [feat.tile([128, 128], F16, tag=f"fglt{c}", name=f"fglt{c}") for c in range(2)]
    FGLR = feat.tile([128, 4096], F16, tag="fglr", name="fglr")  # resized global-attn out
    OUTS = feat.tile([128, 4096], F32, tag="outs", name="outs")
    RD = feat.tile([128, 128], F16, tag="rd", name="rd")
    RDR = feat.tile([128, 128], F32, tag="rdr", name="rdr")
    RDG = feat.tile([32, 32], F16, tag="rdg", name="rdg")
    RDGR = feat.tile([32, 32], F32, tag="rdgr", name="rdgr")
    R4H = feat.tile([4, 4096], F16, tag="r4h", name="r4h")
    R4GH = feat.tile([4, 256], F16, tag="r4gh", name="r4gh")

    ZROW = feat.tile([1, 128], F16, tag="zrow", name="zrow")
    nc.gpsimd.memset(ZROW[:], 0.0)
    OROW = feat.tile([1, 256], F16, tag="orow", name="orow")
    nc.gpsimd.memset(OROW[:], 1.0)
    nc.gpsimd.memset(FLA[64:65, :], 1.0)
    nc.gpsimd.memset(FGA[64:65, :], 1.0)

    pone_cm = tc.tile_pool(name="pone", bufs=2, space="PSUM")
    pone = pone_cm.__enter__()
    # ---- 3x3 conv (local branch) -------------------------------------------
    # XP rows 0-63: zero-padded 66x66 image; rows 64-127: same shifted +1 elem.
    # Pass ky-pair covers taps (ky,0),(ky,1) with K=128; singles cover (ky,2).
    XPv = XP[:].rearrange("p (y x) -> p y x", x=66)

    def emit_conv_half(half):
        pcs = [ppool.tile([64, 1024], F32, tag="ps2", name=f"pconv{half}_{q}")
               for q in range(2)]
        for w in range(6):
            for tloc in range(4):
                t = half * 4 + tloc
                pc = pcs[tloc // 2]
                dst = pc[:, ts_(tloc % 2, 512)].rearrange("p (y x) -> p y x", x=64)
                if w < 3:
                    nc.tensor.matmul(
                        dst, lhsT=DWP[:, ts_(w, 64)],
                        rhs=XPv[:, 8 * t + w:8 * t + w + 8, 0:64],
                        start=(w == 0), stop=False)
                else:
                    nc.tensor.matmul(
                        dst, lhsT=DWS[:, ts_(w - 3, 64)],
                        rhs=XPv[0:64, 8 * t + (w - 3):8 * t + (w - 3) + 8, 2:66],
                        start=False, stop=(w == 5))
        for q in range(2):
            nc.vector.tensor_scalar(
                FLA[0:64, 2048 * half + 1024 * q:][:, 0:1024], pcs[q][:],
                DEPB[:], 0.0, op0=ADD, op1=MAX)

    def emit_q_pair(half):
        pq = ppool.tile([128, 1024], F32, tag="ps2", name=f"pq{half}")
        for cc in range(2):
            nc.tensor.matmul(pq[:, ts_(cc, 512)], lhsT=WQKT[:, 0:128],
                             rhs=FLA[:, 2048 * half + 512 * cc:][:, 0:512],
                             start=True, stop=True)
        nc.vector.tensor_copy(LQ[:, 2048 * half:][:, 0:1024], pq[:])

    def emit_q_pair_b(half):
        pq = ppool.tile([128, 1024], F32, tag="ps2", name=f"pqb{half}")
        for cc in range(2):
            nc.tensor.matmul(pq[:, ts_(cc, 512)], lhsT=WQKT[:, 0:128],
                             rhs=FLA[:, 2048 * half + 1024 + 512 * cc:][:, 0:512],
                             start=True, stop=True)
        nc.vector.tensor_copy(LQ[:, 2048 * half + 1024:][:, 0:1024], pq[:])

    # ---- 4x4 mean pool (global branch) -------------------------------------
    XGh = XG[:].rearrange("p (y x2 two) -> p y x2 two", y=64, x2=32, two=2)          # 64,64,32,2
    PH1 = feat.tile([64, 64, 32], F32, tag="ph1", name="ph1")
    nc.vector.tensor_tensor(PH1[:], XGh[:, :, :, 0], XGh[:, :, :, 1], op=ADD)
    PH1h = PH1[:].rearrange("p y (x2 two) -> p y x2 two", x2=16, two=2)        # 64,64,16,2
    PH2 = feat.tile([64, 64, 16], F32, tag="ph2", name="ph2")
    nc.vector.tensor_tensor(PH2[:], PH1h[:, :, :, 0], PH1h[:, :, :, 1], op=ADD)
    PH2v = PH2[:].rearrange("p (y2 two) x -> p y2 two x", y2=32, two=2)        # 64,32,2,16
    PV1 = feat.tile([64, 32, 16], F32, tag="pv1", name="pv1")
    nc.vector.tensor_tensor(PV1[:], PH2v[:, 0:32, 0, :], PH2v[:, 0:32, 1, :], op=ADD)
    PV1v = PV1[:].rearrange("p (y2 two) x -> p y2 two x", y2=16, two=2)        # 64,16,2,16
    PV2 = feat.tile([64, 16, 16], F32, tag="pv2", name="pv2")
    nc.vector.tensor_tensor(PV2[:], PV1v[:, :, 0, :], PV1v[:, :, 1, :], op=ADD)
    nc.vector.tensor_scalar(
        FGA[0:64, :], PV2[:].rearrange("p y x -> p (y x)"), 1.0 / 16.0, None, op0=MULT)

    if STAGE <= 1:
        _finish(FLA[0:64, :], 64, 4096)
        return
    # ---- local->global attention (Flg) -------------------------------------
    # scores[kg, ql] per head; softmax over kg; E @ gv col-tiled with ones col.
    for j in range(8):
        if j == 0:
            emit_conv_half(0)
            emit_q_pair(0)
            emit_q_pair_b(0)
            # global qkv (pool chain finished on DVE during conv)
            pg = ppool.tile([128, 1024], F32, tag="ps2", name="pqkvg")
            nc.tensor.matmul(pg[:, 0:256], lhsT=WQKT[:, 0:128], rhs=FGA[:],
                             start=True, stop=True)
            nc.tensor.matmul(pg[:, 256:512], lhsT=WQKT[:, 128:256], rhs=FGA[:],
                             start=True, stop=True)
            for c2 in range(2):
                nc.tensor.matmul(pg[:, 512 + 128 * c2:640 + 128 * c2],
                                 lhsT=FGA[:, ts_(c2, 128)], rhs=WVT[:],
                                 start=True, stop=True)
            nc.vector.tensor_copy(GQ[:], pg[:, 0:256])
            nc.vector.tensor_copy(GK[:], pg[:, 256:512])
            nc.vector.tensor_copy(GV[:], pg[:, 512:768])
        elif j == 4:
            emit_conv_half(1)
            emit_q_pair(1)
            emit_q_pair_b(1)
        # k + v chunks for tokens 512j..512j+512 (hidden under exp)
        pkv = ppool.tile([128, 1024], F32, tag="ps2", name="pkv")
        nc.tensor.matmul(pkv[:, 0:512], lhsT=WQKT[:, 128:256],
                         rhs=FLA[:, ts_(j, 512)], start=True, stop=True)
        for cc in range(4):
            nc.tensor.matmul(pkv[:, 512 + 128 * cc:640 + 128 * cc],
                             lhsT=FLA[:, ts_(4 * j + cc, 128)],
                             rhs=WVT[:], start=True, stop=True)
        nc.vector.tensor_copy(LK[:, ts_(j, 512)], pkv[:, 0:512])
        nc.vector.tensor_copy(LV[:, ts_(j, 512)], pkv[:, 512:1024])
        es = []
        for kgh in range(2):
            for p2 in range(2):
                ps = ppool.tile([128, 1024], F32, tag="ps2", name="psc")
                for hh in range(2):
                    h = 2 * p2 + hh
                    nc.tensor.matmul(
                        ps[:, ts_(hh, 512)],
                        lhsT=GK[32 * h:32 * h + 16, ts_(kgh, 128)],
                        rhs=LQ[32 * h:32 * h + 16, ts_(j, 512)],
                        start=True, stop=True, tile_position=(32 * h, 0))
                e = epool.tile([128, 1024], F16, tag="e", name="e")
                nc.scalar.activation(e[:], ps[:], EXP, scale=SCALE)
                es.append(e)
        fp = pone.tile([128, 512], F32, tag="pone", name="pflg")
        for h in range(4):
            for kgh in range(2):
                e = es[kgh * 2 + h // 2]
                nc.tensor.matmul(
                    fp[32 * h:32 * h + 32, :],
                    lhsT=GV[:, 128 * kgh + 32 * h:][:, 0:32],
                    rhs=e[:, ts_(h % 2, 512)],
                    start=(kgh == 0), stop=(kgh == 1), tile_position=(0, 32 * h))
        nc.vector.tensor_copy(FLGR[:, ts_(j, 512)], fp[:])


    if STAGE <= 3:
        _finish(FLGR[:], 128, 4096)
        return
    # ---- global->local attention (Fgl) -------------------------------------
    # scores[lt, qg] per head; softmax over lt; accumulate E @ lv over chunks.
    pone_cm.__exit__(None, None, None)
    pnorm_cm = tc.tile_pool(name="pnorm", bufs=1, space="PSUM")
    pnorm = pnorm_cm.__enter__()
    pacc_cm = tc.tile_pool(name="pacc", bufs=1, space="PSUM")
    pacc = pacc_cm.__enter__()
    fgp = pacc.tile([128, 256], F32, tag="pfgl", name="pfgl")
    if STAGE == 37:
        _finish(GQ[:], 128, 256)
        return
    # pre-zero the accumulator bank and set has_written bits; the per-head
    # col-tiled accumulations then use start=False (overwrite-where-clear).
    nc.tensor.matmul(fgp[:], lhsT=ZROW[:], rhs=OROW[:],
                     start=True, stop=True)
    # lg softmax denominators -> reciprocal -> f16 -> PE broadcast -> normalize
    # (overlapped with the gl loop; pnorm holds the single broadcast bank)
    for h in range(4):
        nc.sync.dma_start(out=RD[32 * h:32 * h + 32, :],
                          in_=FLGR[32 * h + 16:32 * h + 17, :])
    nc.vector.reciprocal(RDR[:], RD[:])
    for h in range(4):
        nc.gpsimd.dma_start(out=R4H[h:h + 1, :], in_=RDR[32 * h:32 * h + 32, :])
    for rc in range(8):
        rb = pnorm.tile([128, 512], F32, tag="pnorm", name="prb")
        nc.tensor.matmul(rb[:], lhsT=SEL4[:], rhs=R4H[:, ts_(rc, 512)],
                         start=True, stop=True)
        nc.vector.tensor_tensor(FLGN[:, ts_(rc, 512)],
                                FLGR[:, ts_(rc, 512)], rb[:], op=MULT)
    for g in range(16):
        psA = ppool.tile([128, 1024], F32, tag="ps2", name="pscA")
        psB = ppool.tile([128, 1024], F32, tag="ps2", name="pscB")
        for ii in range(2):
            i = 2 * g + ii
            for h in range(4):
                pst = psA if h < 2 else psB
                col = 512 * (h % 2) + 256 * ii
                nc.tensor.matmul(
                    pst[:, col:col + 256],
                    lhsT=LK[32 * h:32 * h + 16, ts_(i, 128)],
                    rhs=GQ[32 * h:32 * h + 16, 0:256],
                    start=True, stop=True, tile_position=(32 * h, 0))
        eA = epool.tile([128, 1024], F16, tag="e", name="eA")
        nc.scalar.activation(eA[:], psA[:], EXP, scale=SCALE)
        eB = epool.tile([128, 1024], F16, tag="e", name="eB")
        nc.scalar.activation(eB[:], psB[:], EXP, scale=SCALE)
        for ii in range(2):
            i = 2 * g + ii
            for h in range(4):
                e = eA if h < 2 else eB
                col = 512 * (h % 2) + 256 * ii
                nc.tensor.matmul(
                    fgp[32 * h:32 * h + 32, :],
                    lhsT=LV[:, 128 * i + 32 * h:][:, 0:32],
                    rhs=e[:, col:col + 256],
                    start=False, stop=(i == 31 and h == 3),
                    skip_group_check=True, tile_position=(0, 32 * h))
    if STAGE == 35:
        return
    nc.vector.tensor_copy(FGLS[:], fgp[:])
    pacc_cm.__exit__(None, None, None)
    if STAGE <= 4:
        _finish(FGLS[:], 128, 256)
        return

    for h in range(4):
        nc.sync.dma_start(out=RDG[8 * h:8 * h + 8, :],
                          in_=FGLS[32 * h + 16:32 * h + 17, :])
    nc.vector.reciprocal(RDGR[:], RDG[:])
    for h in range(4):
        nc.gpsimd.dma_start(out=R4GH[h:h + 1, :], in_=RDGR[8 * h:8 * h + 8, :])
    rbg = pnorm.tile([128, 256], F32, tag="pnorm", name="prbg")
    nc.tensor.matmul(rbg[:], lhsT=SEL4[:], rhs=R4GH[:], start=True, stop=True)
    nc.vector.tensor_tensor(FGLN[:], FGLS[:], rbg[:], op=MULT)
    pnorm_cm.__exit__(None, None, None)
    ppool_cm.__exit__(None, None, None)
    ptail_cm = tc.tile_pool(name="ptail", bufs=2, space="PSUM")
    ptail = ptail_cm.__enter__()

    # ---- bilinear resize: transpose then dense resize matmul ----------------
    for c in range(2):
        pt = ptail.tile([128, 128], F16, tag="ptail", name="ptr")
        nc.tensor.transpose(pt[:], FGLN[:, ts_(c, 128)], IDENT[:])
        nc.vector.tensor_copy(FGLT[c][:], pt[:])
    for n4 in range(4):
        pr = ptail.tile([128, 1024], F32, tag="ptail", name="prz")
        for half in range(2):
            n2 = 2 * n4 + half
            for c in range(2):
                nc.tensor.matmul(pr[:, ts_(half, 512)], lhsT=FGLT[c][:],
                                 rhs=R2[c][:, ts_(n2, 512)],
                                 start=(c == 0), stop=(c == 1))
        nc.vector.tensor_copy(FGLR[:, ts_(n4, 1024)], pr[:])

    if STAGE <= 5:
        _finish(FGLR[:], 128, 4096)
        return
    # ---- 1x1 conv + bias + relu --------------------------------------------
    for n4 in range(4):
        po = ptail.tile([128, 1024], F32, tag="ptail", name="pout")
        for half in range(2):
            n2 = 2 * n4 + half
            nc.tensor.matmul(po[:, ts_(half, 512)], lhsT=LWT[:, 0:128],
                             rhs=FLGN[:, ts_(n2, 512)], start=True, stop=False)
            nc.tensor.matmul(po[:, ts_(half, 512)], lhsT=LWT[:, 128:256],
                             rhs=FGLR[:, ts_(n2, 512)], start=False, stop=True)
        nc.vector.tensor_scalar(OUTS[:, ts_(n4, 1024)], po[:], LB[:], 0.0,
                                op0=ADD, op1=MAX)
        nc.sync.dma_start(out=d["out"][:, 1024 * n4:1024 * (n4 + 1)],
                          in_=OUTS[:, ts_(n4, 1024)])
    ptail_cm.__exit__(None, None, None)


def _build():
    nc = bacc.Bacc("TRN2", target_bir_lowering=False, debug=False)
    d = {}
    specs = [
        ("xp", [128, 66 * 66], F16), ("xg", [64, 4096], F16),
        ("dwp", [128, 192], F16), ("dws", [64, 192], F16), ("depb", [64, 1], F32),
        ("wqkt", [65, 256], F16), ("wvt", [65, 128], F16),
        ("lwt", [128, 256], F16), ("lb", [128, 1], F32),
        ("r2dt", [256, 4096], F16), ("sel4", [4, 128], F16),
        ("ident", [128, 128], F16),
    ]
    for name, shape, dt in specs:
        d[name] = nc.dram_tensor(name, shape, dt, kind="ExternalInput").ap()
    d["out"] = nc.dram_tensor("out", [128, 4096], F32, kind="ExternalOutput").ap()

    with tile.TileContext(nc) as tc:
        with ExitStack() as ctx:
            _emit(ctx, tc, nc, d)
    nc.compile()
    return nc


_CACHE = {}


def _prep_shared(dep_w, dep_scale, dep_bias, qkv_w, qkv_b, l_w, l_scale, l_bias):
    f16 = np.float16
    f32 = np.float32
    dw = (dep_w * dep_scale[:, None, None, None]).astype(f32)   # [co, ci, 3, 3]
    dwp = np.zeros((128, 192), f16)
    dws = np.zeros((64, 192), f16)
    for ky in range(3):
        dwp[0:64, 64 * ky:64 * (ky + 1)] = dw[:, :, ky, 0].T
        dwp[64:128, 64 * ky:64 * (ky + 1)] = dw[:, :, ky, 1].T
        dws[:, 64 * ky:64 * (ky + 1)] = dw[:, :, ky, 2].T

    wqkt = np.zeros((65, 256), f16)
    wvt = np.zeros((65, 128), f16)
    for h in range(4):
        for dd in range(16):
            ch = 16 * h + dd
            wqkt[0:64, 32 * h + dd] = qkv_w[ch, :]
            wqkt[64, 32 * h + dd] = qkv_b[ch]
            wqkt[0:64, 128 + 32 * h + dd] = qkv_w[64 + ch, :]
            wqkt[64, 128 + 32 * h + dd] = qkv_b[64 + ch]
            wvt[0:64, 32 * h + dd] = qkv_w[128 + ch, :]
            wvt[64, 32 * h + dd] = qkv_b[128 + ch]
        wvt[64, 32 * h + 16] = 1.0   # ones column -> softmax denominator

    lw = (l_w[:, :, 0, 0] * l_scale[:, None]).astype(f32)       # [co, cin]
    lwt = np.zeros((128, 256), f16)
    for h in range(4):
        for dd in range(16):
            lwt[32 * h + dd, 0:128] = lw[:, 16 * h + dd]
            lwt[32 * h + dd, 128:256] = lw[:, 64 + 16 * h + dd]

    R1 = _resize_matrix()
    r2d = np.kron(R1, R1)                                        # [4096, 256]
    r2dt = np.ascontiguousarray(r2d.T).astype(f16)               # [256, 4096]

    sel4 = np.zeros((4, 128), f16)
    for h in range(4):
        sel4[h, 32 * h:32 * h + 32] = 1.0

    return {
        "dwp": dwp, "dws": dws, "depb": dep_bias.reshape(64, 1).astype(f32),
        "wqkt": wqkt, "wvt": wvt, "lwt": lwt,
        "lb": l_bias.reshape(128, 1).astype(f32),
        "r2dt": r2dt, "sel4": sel4,
        "ident": np.eye(128, dtype=f16),
    }


def build_in_maps(inputs, dep_w, dep_scale, dep_bias, qkv_w, qkv_b, l_w, l_scale,
                  l_bias):
    shared = _prep_shared(np.asarray(dep_w, np.float32), np.asarray(dep_scale, np.float32),
                          np.asarray(dep_bias, np.float32), np.asarray(qkv_w, np.float32),
                          np.asarray(qkv_b, np.float32), np.asarray(l_w, np.float32),
                          np.asarray(l_scale, np.float32), np.asarray(l_bias, np.float32))
    x = np.asarray(inputs, np.float32)
    in_maps = []
    for b in range(B):
        xp = np.zeros((128, 66, 66), np.float16)
        xp[0:64, 1:65, 1:65] = x[b, 0:64]
        xp[64:128, 1:65, 0:64] = x[b, 0:64]   # shifted +1 element copy
        m = dict(shared)
        m["xp"] = xp.reshape(128, 66 * 66)
        m["xg"] = np.ascontiguousarray(x[b, 64:128].reshape(64, 4096)).astype(np.float16)
        in_maps.append(m)
    return in_maps


def get_program():
    if "nc" not in _CACHE:
        _CACHE["nc"] = _build()
    return _CACHE["nc"]


def kernel(inputs, dep_w, dep_scale, dep_bias, qkv_w, qkv_b, l_w, l_scale, l_bias):
    nc = get_program()
    in_maps = build_in_maps(inputs, dep_w, dep_scale, dep_bias, qkv_w, qkv_b,
                            l_w, l_scale, l_bias)
    res = run_bass_kernel_spmd(nc, in_maps, core_ids=list(range(B)))
    out = np.stack([r["out"].reshape(C, H, W) for r in res.results])
    return out.astype(np.float32)

